# revision 19
# baseline (speedup 1.0000x reference)
"""Trainium2 Bass kernel for nn_Ensemble_FC (BatchEnsemble fully-connected layer).

Math (reference):
    emb   = relu(alpha @ enc1_w.T + enc1_b)          # (M, H)
    mu    = emb @ encm_w.T + encm_b                  # (M, H)
    z     = eps * exp(0.5 * mu) + mu
    adec  = z @ dec_w.T + dec_b                      # (M, IN)
    out[m*B+i, o] = (sum_k x[i,k] * adec[m,k] * fc_w[o,k]) * gamma[m,o] + bias_p[m,o]

The VAE encoder (~1M MACs, 0.003% of total FLOPs) runs on the HOST in f32,
and the per-model scale is folded into the weights on the host:
w'[m] = fc_w ⊙ adec[m] (bf16).  The device kernel is then a pure streamed
GEMM with NO per-matmul vector work — the PE is the only serial resource.

Sharding: tensor-parallel column-split of fc_w / gamma / bias_p over
out_features (4096 -> 8 x 512).  Every core computes the full
(M*B = 2048)-row GEMM for its 512 output columns:
    out_core[o_local, m*B+i] = psum * gamma + bias,
    psum = sum_kc  w'[m][kc, o-chunk].T @ xT[kc]

Perf structure (trace-driven):
- ~7us fixed runtime prologue before any instruction, ~11.5us of fixed
  exec-window overhead outside the instruction span.
- PE warm-up matmuls bridge the first bulk-DMA group's latency and trip
  the HAM clock gate (cold PE runs at 1.2 GHz).
- Streams: x (4MB) on the sync HWDGE ring, w'[m0,m1] (8MB) on the Pool
  ring — both at ~179 GB/s concurrently (= the 358 GB/s HBM roofline).
  w'[m2,m3] (8MB) is gated behind the pass-A weights' completion so the
  early 2-way split isn't diluted to 3 queues.
- Pass A (m0,m1 x 4 o-chunks, k-outer over arrival order) tracks the
  DMA rate; pass B (m2,m3) runs on resident x.  Both passes run
  k=0..27 for all 8 PSUM groups, then finish each group's k=28..31
  group-major so completions stagger and the epilogue/store tail and
  the A->B PSUM-bank handoff pipeline instead of bunching.
- Output stores alternate between the vector and sync rings.
"""

import os
import sys

for _p in ("/opt/trn_rl_repo",):
    if os.path.isdir(_p) and _p not in sys.path:
        sys.path.insert(0, _p)

import numpy as np
import ml_dtypes

import concourse.bass as bass  # noqa: F401  (registers engine libraries)
import concourse.mybir as mybir
import concourse.tile as tile
from concourse import bacc
from concourse.bass_utils import run_bass_kernel_spmd

N_CORES = 8
M = 4          # ensemble members
B = 512        # batch
IN = 4096      # in_features (contraction)
OUT = 4096     # out_features
H = 32         # encoder hidden
P = 128        # partitions
KC = IN // P   # 32 contraction chunks of 128
O_CORE = OUT // N_CORES   # 512 output columns per core
OC = O_CORE // P          # 4 o-chunks of 128 per core
N_WARM = 8     # PE warm-up matmuls
K_TAIL = 8     # per-group staggered tail length (k = KC-K_TAIL .. KC-1)

# bulk-stream DMA groups (kc each); small head groups so the first
# matmuls aren't gated on a big first transfer.  x uses groups twice
# the size of w's: the SDMA engines round-robin per PACKET, and a
# packet is one per-partition contiguous run (kcs * 1KB for x,
# kcs * 2KB for w) — matching packet sizes splits bandwidth 50/50
# between the two streams instead of starving x 2:1.
W_GROUP_KCS = [1, 1, 2, 4, 4, 4, 4, 4, 4, 4]
X_GROUP_KCS = [1, 1, 2, 4, 4, 4, 4, 4, 4, 4]
X_HEAD_SCALAR = 3   # first x groups ride the idle ACT ring for a fast start


def _group_maps(kcs):
    of_k = []
    for g, n in enumerate(kcs):
        of_k += [(g, j) for j in range(n)]
    k0 = [sum(kcs[:g]) for g in range(len(kcs))]
    return of_k, k0


W_OF_K, W_K0 = _group_maps(W_GROUP_KCS)
X_OF_K, X_K0 = _group_maps(X_GROUP_KCS)
GW = len(W_GROUP_KCS)
GX = len(X_GROUP_KCS)

# gb32 column layout (f32, [128, GB_W])
GB_G = 0                      # [p, oc, m]  OC*M = 16
GB_B = GB_G + OC * M
GB_W = GB_B + OC * M          # 32

F32 = mybir.dt.float32
BF16 = mybir.dt.bfloat16
AF = mybir.ActivationFunctionType

_nc_cache = {}


def _build_nc():
    """Build and compile the per-core Bass/Tile program (SPMD, same on all 8)."""
    nc = bacc.Bacc("TRN2", num_devices=N_CORES, debug=False)

    xh_d = nc.declare_dram_parameter("xh", [P, KC, B], BF16, isOutput=False)
    wa_d = nc.declare_dram_parameter("wa", [P, KC, 2, O_CORE], BF16, isOutput=False)
    wb_d = nc.declare_dram_parameter("wb", [P, KC, 2, O_CORE], BF16, isOutput=False)
    gb32_d = nc.declare_dram_parameter("gb32", [P, GB_W], F32, isOutput=False)
    out_d = nc.declare_dram_parameter("out", [O_CORE, M * B], F32, isOutput=True)

    with tile.TileContext(nc) as tc:
        xn_head = sum(1 for k in X_GROUP_KCS if k < max(X_GROUP_KCS))
        wn_head = sum(1 for k in W_GROUP_KCS if k < max(W_GROUP_KCS))
        with (
            tc.tile_pool(name="consts", bufs=1) as consts,
            tc.tile_pool(name="xth", bufs=xn_head) as xth_pool,
            tc.tile_pool(name="xtm", bufs=GX - xn_head) as xtm_pool,
            tc.tile_pool(name="wh", bufs=wn_head) as wh_pool,
            tc.tile_pool(name="wm", bufs=GW - wn_head) as wm_pool,
            tc.tile_pool(name="ps", bufs=8, space="PSUM") as ps_pool,
            tc.tile_pool(name="osb", bufs=8) as out_pool,
        ):
            def x_tile(g):
                pool, tag = (xth_pool, "xth") if g < xn_head else (xtm_pool, "xtm")
                return pool.tile(
                    [P, X_GROUP_KCS[g], B], BF16, tag=tag, name=f"xt_{g}"
                )

            w_n = [0]

            def w_tile(g):
                # wa and wb share the pools: wb group i reuses wa group i's
                # slot once pass A has consumed it (sizes line up by order)
                pool, tag = (wh_pool, "wh") if g < wn_head else (wm_pool, "wm")
                w_n[0] += 1
                return pool.tile(
                    [P, W_GROUP_KCS[g], 2, O_CORE], BF16, tag=tag,
                    name=f"w_{w_n[0]}_{g}",
                )
            # ---- PE warm-up: garbage matmuls bridge the bulk-DMA latency
            # and trip the HAM activity monitor (1.2 GHz -> full rate).
            wu_src = consts.tile([P, B], BF16)
            nc.gpsimd.memset(wu_src[:], 0.0)

            wu_ps = ps_pool.tile([P, B], F32, tag="ps")
            for i in range(N_WARM):
                nc.tensor.matmul(
                    wu_ps[:], lhsT=wu_src[:, :P], rhs=wu_src[:], start=True, stop=True
                )

            # ---- DMA issue.  gb32 is tiny and not needed until the first
            # epilogue (~60us in) — no gating anywhere.  x and wa stream
            # concurrently on two rings; wb is held behind wa's last group
            # so the early phase keeps a 2-way (full-rate) split.
            gb32_sb = consts.tile([P, GB_W], F32)
            nc.scalar.dma_start(gb32_sb[:], gb32_d.ap())

            xt_tiles = []
            for g in range(GX):
                ks = slice(X_K0[g], X_K0[g] + X_GROUP_KCS[g])
                xt = x_tile(g)
                xeng = nc.scalar if g < X_HEAD_SCALAR else nc.sync
                xeng.dma_start(xt[:], xh_d.ap()[:, ks, :])
                xt_tiles.append(xt)
            wa_tiles = []
            wa_last_dma = None
            for g in range(GW):
                ks = slice(W_K0[g], W_K0[g] + W_GROUP_KCS[g])
                wt = w_tile(g)
                wa_last_dma = nc.gpsimd.dma_start(wt[:], wa_d.ap()[:, ks, :, :])
                wa_tiles.append(wt)
            wb_tiles = []
            for g in range(GW):
                ks = slice(W_K0[g], W_K0[g] + W_GROUP_KCS[g])
                wt = w_tile(g)
                wdma = nc.sync.dma_start(wt[:], wb_d.ap()[:, ks, :, :])
                if g == 0:
                    tile.add_dep_helper(
                        wdma.ins, wa_last_dma.ins, reason="wb after wa done"
                    )
                wb_tiles.append(wt)

            g_v = gb32_sb[:, GB_G:GB_B].rearrange("p (o m) -> p o m", m=M)
            b_v = gb32_sb[:, GB_B:GB_W].rearrange("p (o m) -> p o m", m=M)

            # consume the warm-up psum so bacc DCE keeps the warm-up.
            wu_sink = consts.tile([P, B], F32)
            nc.vector.tensor_copy(wu_sink[:], wu_ps[:])

            store_n = [0]

            def epilogue(ps, oc, m, name, engs):
                osb = out_pool.tile([P, B], F32, tag="osb", name=name)
                nc.scalar.activation(
                    osb[:],
                    ps[:],
                    AF.Identity,
                    bias=b_v[:, oc, m : m + 1],
                    scale=g_v[:, oc, m : m + 1],
                )
                eng = engs[store_n[0] % len(engs)]
                store_n[0] += 1
                eng.dma_start(
                    out_d.ap()[oc * P : (oc + 1) * P, m * B : (m + 1) * B],
                    osb[:],
                )

            def gemm_pass(w_tiles, ms, tag, store_engs):
                ps = {
                    (mi, oc): ps_pool.tile(
                        [P, B], F32, tag="ps", name=f"ps{tag}_{mi}_{oc}"
                    )
                    for mi in range(2)
                    for oc in range(OC)
                }
                def mm(k, mi, oc, start, stop):
                    wg, wj = W_OF_K[k]
                    xg, xj = X_OF_K[k]
                    nc.tensor.matmul(
                        ps[(mi, oc)][:],
                        lhsT=w_tiles[wg][:, wj, mi, oc * P : (oc + 1) * P],
                        rhs=xt_tiles[xg][:, xj, :],
                        start=start,
                        stop=stop,
                    )

                for k in range(KC - K_TAIL):
                    for mi in range(2):
                        for oc in range(OC):
                            mm(k, mi, oc, k == 0, False)
                # staggered tail: finish each group's last K_TAIL k-chunks
                # group-major so completions (and PSUM-bank frees) pipeline
                for mi in range(2):
                    for oc in range(OC):
                        for k in range(KC - K_TAIL, KC):
                            mm(k, mi, oc, False, k == KC - 1)
                        m = ms[mi]
                        epilogue(ps[(mi, oc)], oc, m, f"osb{tag}_{mi}_{oc}", store_engs)

            # pass-A stores ride the Pool ring (its wa transfers drain first,
            # and the SP ring is busy with wb); pass-B stores ride SP (idle
            # and fast by then — keeps the final store tail short)
            gemm_pass(wa_tiles, (0, 1), "A", (nc.gpsimd,))
            gemm_pass(wb_tiles, (2, 3), "B", (nc.sync,))

    nc.compile()
    return nc


def _get_nc():
    if "nc" not in _nc_cache:
        _nc_cache["nc"] = _build_nc()
    return _nc_cache["nc"]


def _pk(a2d):
    """(C*P, W) -> (P, C*W): row 128c+p -> [p, c, :] flattened."""
    c = a2d.shape[0] // P
    w = a2d.shape[1]
    return np.ascontiguousarray(
        a2d.reshape(c, P, w).transpose(1, 0, 2).reshape(P, c * w)
    )


def kernel(
    x, eps, alpha, gamma, bias_p, fc_w,
    enc1_w, enc1_b, encm_w, encm_b, dec_w, dec_b,
):
    bf16 = ml_dtypes.bfloat16
    f32 = np.float32
    asc = np.ascontiguousarray

    x = np.asarray(x, f32)
    fc_w = np.asarray(fc_w, f32)

    # ---- VAE encoder on host (f32): adec = dec(reparam(enc(alpha)))
    alpha_f = np.asarray(alpha, f32)
    emb = np.maximum(alpha_f @ np.asarray(enc1_w, f32).T + np.asarray(enc1_b, f32), 0.0)
    mu = emb @ np.asarray(encm_w, f32).T + np.asarray(encm_b, f32)
    z = np.asarray(eps, f32) * np.exp(0.5 * mu) + mu
    adec = (z @ np.asarray(dec_w, f32).T + np.asarray(dec_b, f32)).astype(f32)  # (M, IN)

    # x: (B, IN) -> xh (P, KC, B) bf16, xh[p,k,r] = x[r, 128k+p]
    xh = asc(x.astype(bf16).T.reshape(KC, P, B).transpose(1, 0, 2))
    wT_full = fc_w.T  # (IN, OUT) f32 view

    gT_full = np.asarray(gamma, f32).T                    # (OUT, M)
    bT_full = np.asarray(bias_p, f32).T                   # (OUT, M)

    in_maps = []
    for c in range(N_CORES):
        o0, o1 = c * O_CORE, (c + 1) * O_CORE
        wcore = wT_full[:, o0:o1]  # (IN, O_CORE) f32
        # w'[m] = fc_w * adec[m], folded on host; [p, kc, mi, o] layout
        wm = [
            (wcore * adec[m][:, None]).astype(bf16).reshape(KC, P, O_CORE)
            for m in range(M)
        ]
        wa = asc(np.stack(wm[0:2], axis=2).transpose(1, 0, 2, 3))
        wb = asc(np.stack(wm[2:4], axis=2).transpose(1, 0, 2, 3))
        gb32 = np.empty((P, GB_W), f32)
        gb32[:, GB_G:GB_B] = _pk(asc(gT_full[o0:o1]))
        gb32[:, GB_B:GB_W] = _pk(asc(bT_full[o0:o1]))
        in_maps.append({"xh": xh, "wa": wa, "wb": wb, "gb32": gb32})

    nc = _get_nc()
    res = None
    for attempt in range(3):
        try:
            res = run_bass_kernel_spmd(nc, in_maps, list(range(N_CORES)))
            break
        except Exception:
            # transient NRT_EXEC_UNIT_UNRECOVERABLE wedges can follow an
            # earlier crashed process on the same cores; retry clears it
            if attempt == 2:
                raise
            import time

            time.sleep(5.0)
    outT = np.concatenate(
        [res.results[c]["out"] for c in range(N_CORES)], axis=0
    )  # (OUT, M*B)
    return asc(outT.T.astype(np.float32))  # (M*B, OUT)


# revision 21
# speedup vs baseline: 1.0908x; 1.0908x over previous
"""Trainium2 Bass kernel for nn_Ensemble_FC (BatchEnsemble fully-connected layer).

Math (reference):
    emb   = relu(alpha @ enc1_w.T + enc1_b)          # (M, H)
    mu    = emb @ encm_w.T + encm_b                  # (M, H)
    z     = eps * exp(0.5 * mu) + mu
    adec  = z @ dec_w.T + dec_b                      # (M, IN)
    out[m*B+i, o] = (sum_k x[i,k] * adec[m,k] * fc_w[o,k]) * gamma[m,o] + bias_p[m,o]

The VAE encoder (~1M MACs, 0.003% of total FLOPs) runs on the HOST in f32,
and the per-model scale is folded into the weights on the host:
w'[m] = fc_w ⊙ adec[m] (bf16).  The device kernel is then a pure streamed
GEMM with NO per-matmul vector work — the PE is the only serial resource.

Sharding: tensor-parallel column-split of fc_w / gamma / bias_p over
out_features (4096 -> 8 x 512).  Every core computes the full
(M*B = 2048)-row GEMM for its 512 output columns:
    out_core[o_local, m*B+i] = psum * gamma + bias,
    psum = sum_kc  w'[m][kc, o-chunk].T @ xT[kc]

Perf structure (trace-driven):
- ~7us fixed runtime prologue before any instruction, ~11.5us of fixed
  exec-window overhead outside the instruction span.
- PE warm-up matmuls bridge the first bulk-DMA group's latency and trip
  the HAM clock gate (cold PE runs at 1.2 GHz).
- Streams: x (4MB) on the sync HWDGE ring, w'[m0,m1] (8MB) on the Pool
  ring — both at ~179 GB/s concurrently (= the 358 GB/s HBM roofline).
  w'[m2,m3] (8MB) is gated behind the pass-A weights' completion so the
  early 2-way split isn't diluted to 3 queues.
- Pass A (m0,m1 x 4 o-chunks, k-outer over arrival order) tracks the
  DMA rate; pass B (m2,m3) runs on resident x.  Both passes run
  k=0..27 for all 8 PSUM groups, then finish each group's k=28..31
  group-major so completions stagger and the epilogue/store tail and
  the A->B PSUM-bank handoff pipeline instead of bunching.
- Output stores alternate between the vector and sync rings.
"""

import os
import sys

for _p in ("/opt/trn_rl_repo",):
    if os.path.isdir(_p) and _p not in sys.path:
        sys.path.insert(0, _p)

import numpy as np
import ml_dtypes

import concourse.bass as bass  # noqa: F401  (registers engine libraries)
import concourse.mybir as mybir
import concourse.tile as tile
from concourse import bacc
from concourse.bass_utils import run_bass_kernel_spmd

N_CORES = 8
M = 4          # ensemble members
B = 512        # batch
IN = 4096      # in_features (contraction)
OUT = 4096     # out_features
H = 32         # encoder hidden
P = 128        # partitions
KC = IN // P   # 32 contraction chunks of 128
O_CORE = OUT // N_CORES   # 512 output columns per core
OC = O_CORE // P          # 4 o-chunks of 128 per core
N_WARM = 8     # PE warm-up matmuls
K_TAIL = 8     # per-group staggered tail length (k = KC-K_TAIL .. KC-1)

# bulk-stream DMA groups (kc each); small head groups so the first
# matmuls aren't gated on a big first transfer.  x uses groups twice
# the size of w's: the SDMA engines round-robin per PACKET, and a
# packet is one per-partition contiguous run (kcs * 1KB for x,
# kcs * 2KB for w) — matching packet sizes splits bandwidth 50/50
# between the two streams instead of starving x 2:1.
W_GROUP_KCS = [1, 1, 2, 4, 4, 4, 4, 4, 4, 4]
X_GROUP_KCS = [1, 1, 2, 4, 4, 4, 4, 4, 4, 4]


def _group_maps(kcs):
    of_k = []
    for g, n in enumerate(kcs):
        of_k += [(g, j) for j in range(n)]
    k0 = [sum(kcs[:g]) for g in range(len(kcs))]
    return of_k, k0


W_OF_K, W_K0 = _group_maps(W_GROUP_KCS)
X_OF_K, X_K0 = _group_maps(X_GROUP_KCS)
GW = len(W_GROUP_KCS)
GX = len(X_GROUP_KCS)

# gb32 column layout (f32, [128, GB_W])
GB_G = 0                      # [p, oc, m]  OC*M = 16
GB_B = GB_G + OC * M
GB_W = GB_B + OC * M          # 32

F32 = mybir.dt.float32
BF16 = mybir.dt.bfloat16
AF = mybir.ActivationFunctionType

_nc_cache = {}


def _build_nc():
    """Build and compile the per-core Bass/Tile program (SPMD, same on all 8)."""
    nc = bacc.Bacc("TRN2", num_devices=N_CORES, debug=False)

    xh_d = nc.declare_dram_parameter("xh", [P, KC, B], BF16, isOutput=False)
    wa_d = nc.declare_dram_parameter("wa", [P, KC, 2, O_CORE], BF16, isOutput=False)
    wb_d = nc.declare_dram_parameter("wb", [P, KC, 2, O_CORE], BF16, isOutput=False)
    gb32_d = nc.declare_dram_parameter("gb32", [P, GB_W], F32, isOutput=False)
    out_d = nc.declare_dram_parameter("out", [O_CORE, M * B], F32, isOutput=True)

    with tile.TileContext(nc) as tc:
        xn_head = sum(1 for k in X_GROUP_KCS if k < max(X_GROUP_KCS))
        wn_head = sum(1 for k in W_GROUP_KCS if k < max(W_GROUP_KCS))
        with (
            tc.tile_pool(name="consts", bufs=1) as consts,
            tc.tile_pool(name="xth", bufs=xn_head) as xth_pool,
            tc.tile_pool(name="xtm", bufs=GX - xn_head) as xtm_pool,
            tc.tile_pool(name="wh", bufs=wn_head) as wh_pool,
            tc.tile_pool(name="wm", bufs=GW - wn_head) as wm_pool,
            tc.tile_pool(name="ps", bufs=8, space="PSUM") as ps_pool,
            tc.tile_pool(name="osb", bufs=8) as out_pool,
        ):
            def x_tile(g):
                pool, tag = (xth_pool, "xth") if g < xn_head else (xtm_pool, "xtm")
                return pool.tile(
                    [P, X_GROUP_KCS[g], B], BF16, tag=tag, name=f"xt_{g}"
                )

            w_n = [0]

            def w_tile(g):
                # wa and wb share the pools: wb group i reuses wa group i's
                # slot once pass A has consumed it (sizes line up by order)
                pool, tag = (wh_pool, "wh") if g < wn_head else (wm_pool, "wm")
                w_n[0] += 1
                return pool.tile(
                    [P, W_GROUP_KCS[g], 2, O_CORE], BF16, tag=tag,
                    name=f"w_{w_n[0]}_{g}",
                )
            # ---- PE warm-up: garbage matmuls bridge the bulk-DMA latency
            # and trip the HAM activity monitor (1.2 GHz -> full rate).
            wu_src = consts.tile([P, B], BF16)
            nc.gpsimd.memset(wu_src[:], 0.0)

            wu_ps = ps_pool.tile([P, B], F32, tag="ps")
            for i in range(N_WARM):
                nc.tensor.matmul(
                    wu_ps[:], lhsT=wu_src[:, :P], rhs=wu_src[:], start=True, stop=True
                )

            # ---- DMA issue.  gb32 is tiny and not needed until the first
            # epilogue (~60us in) — no gating anywhere.  x and wa stream
            # concurrently on two rings; wb is held behind wa's last group
            # so the early phase keeps a 2-way (full-rate) split.
            gb32_sb = consts.tile([P, GB_W], F32)
            nc.scalar.dma_start(gb32_sb[:], gb32_d.ap())

            xt_tiles = []
            for g in range(GX):
                ks = slice(X_K0[g], X_K0[g] + X_GROUP_KCS[g])
                xt = x_tile(g)
                nc.sync.dma_start(xt[:], xh_d.ap()[:, ks, :])
                xt_tiles.append(xt)
            wa_tiles = []
            wa_last_dma = None
            for g in range(GW):
                ks = slice(W_K0[g], W_K0[g] + W_GROUP_KCS[g])
                wt = w_tile(g)
                wa_last_dma = nc.gpsimd.dma_start(wt[:], wa_d.ap()[:, ks, :, :])
                wa_tiles.append(wt)
            wb_tiles = []
            for g in range(GW):
                ks = slice(W_K0[g], W_K0[g] + W_GROUP_KCS[g])
                wt = w_tile(g)
                wdma = nc.sync.dma_start(wt[:], wb_d.ap()[:, ks, :, :])
                if g == 0:
                    tile.add_dep_helper(
                        wdma.ins, wa_last_dma.ins, reason="wb after wa done"
                    )
                wb_tiles.append(wt)

            g_v = gb32_sb[:, GB_G:GB_B].rearrange("p (o m) -> p o m", m=M)
            b_v = gb32_sb[:, GB_B:GB_W].rearrange("p (o m) -> p o m", m=M)

            # consume the warm-up psum so bacc DCE keeps the warm-up.
            wu_sink = consts.tile([P, B], F32)
            nc.vector.tensor_copy(wu_sink[:], wu_ps[:])

            store_n = [0]

            def epilogue(ps, oc, m, name, engs):
                osb = out_pool.tile([P, B], F32, tag="osb", name=name)
                nc.scalar.activation(
                    osb[:],
                    ps[:],
                    AF.Identity,
                    bias=b_v[:, oc, m : m + 1],
                    scale=g_v[:, oc, m : m + 1],
                )
                eng = engs[store_n[0] % len(engs)]
                store_n[0] += 1
                eng.dma_start(
                    out_d.ap()[oc * P : (oc + 1) * P, m * B : (m + 1) * B],
                    osb[:],
                )

            def gemm_pass(w_tiles, ms, tag, store_engs):
                ps = {
                    (mi, oc): ps_pool.tile(
                        [P, B], F32, tag="ps", name=f"ps{tag}_{mi}_{oc}"
                    )
                    for mi in range(2)
                    for oc in range(OC)
                }
                def mm(k, mi, oc, start, stop):
                    wg, wj = W_OF_K[k]
                    xg, xj = X_OF_K[k]
                    nc.tensor.matmul(
                        ps[(mi, oc)][:],
                        lhsT=w_tiles[wg][:, wj, mi, oc * P : (oc + 1) * P],
                        rhs=xt_tiles[xg][:, xj, :],
                        start=start,
                        stop=stop,
                    )

                for k in range(KC - K_TAIL):
                    for mi in range(2):
                        for oc in range(OC):
                            mm(k, mi, oc, k == 0, False)
                # staggered tail: finish each group's last K_TAIL k-chunks
                # group-major so completions (and PSUM-bank frees) pipeline
                for mi in range(2):
                    for oc in range(OC):
                        for k in range(KC - K_TAIL, KC):
                            mm(k, mi, oc, False, k == KC - 1)
                        m = ms[mi]
                        epilogue(ps[(mi, oc)], oc, m, f"osb{tag}_{mi}_{oc}", store_engs)

            # pass-A stores ride the Pool ring (its wa transfers drain first,
            # and the SP ring is busy with wb); pass-B stores ride SP (idle
            # and fast by then — keeps the final store tail short)
            gemm_pass(wa_tiles, (0, 1), "A", (nc.gpsimd,))
            gemm_pass(wb_tiles, (2, 3), "B", (nc.sync,))

    nc.compile()
    return nc


def _get_nc():
    if "nc" not in _nc_cache:
        _nc_cache["nc"] = _build_nc()
    return _nc_cache["nc"]


def _pk(a2d):
    """(C*P, W) -> (P, C*W): row 128c+p -> [p, c, :] flattened."""
    c = a2d.shape[0] // P
    w = a2d.shape[1]
    return np.ascontiguousarray(
        a2d.reshape(c, P, w).transpose(1, 0, 2).reshape(P, c * w)
    )


def kernel(
    x, eps, alpha, gamma, bias_p, fc_w,
    enc1_w, enc1_b, encm_w, encm_b, dec_w, dec_b,
):
    bf16 = ml_dtypes.bfloat16
    f32 = np.float32
    asc = np.ascontiguousarray

    x = np.asarray(x, f32)
    fc_w = np.asarray(fc_w, f32)

    # ---- VAE encoder on host (f32): adec = dec(reparam(enc(alpha)))
    alpha_f = np.asarray(alpha, f32)
    emb = np.maximum(alpha_f @ np.asarray(enc1_w, f32).T + np.asarray(enc1_b, f32), 0.0)
    mu = emb @ np.asarray(encm_w, f32).T + np.asarray(encm_b, f32)
    z = np.asarray(eps, f32) * np.exp(0.5 * mu) + mu
    adec = (z @ np.asarray(dec_w, f32).T + np.asarray(dec_b, f32)).astype(f32)  # (M, IN)

    # x: (B, IN) -> xh (P, KC, B) bf16, xh[p,k,r] = x[r, 128k+p]
    xh = asc(x.astype(bf16).T.reshape(KC, P, B).transpose(1, 0, 2))
    wT_full = fc_w.T  # (IN, OUT) f32 view

    gT_full = np.asarray(gamma, f32).T                    # (OUT, M)
    bT_full = np.asarray(bias_p, f32).T                   # (OUT, M)

    in_maps = []
    for c in range(N_CORES):
        o0, o1 = c * O_CORE, (c + 1) * O_CORE
        wcore = wT_full[:, o0:o1]  # (IN, O_CORE) f32
        # w'[m] = fc_w * adec[m], folded on host; [p, kc, mi, o] layout
        wm = [
            (wcore * adec[m][:, None]).astype(bf16).reshape(KC, P, O_CORE)
            for m in range(M)
        ]
        wa = asc(np.stack(wm[0:2], axis=2).transpose(1, 0, 2, 3))
        wb = asc(np.stack(wm[2:4], axis=2).transpose(1, 0, 2, 3))
        gb32 = np.empty((P, GB_W), f32)
        gb32[:, GB_G:GB_B] = _pk(asc(gT_full[o0:o1]))
        gb32[:, GB_B:GB_W] = _pk(asc(bT_full[o0:o1]))
        in_maps.append({"xh": xh, "wa": wa, "wb": wb, "gb32": gb32})

    nc = _get_nc()
    res = None
    for attempt in range(3):
        try:
            res = run_bass_kernel_spmd(nc, in_maps, list(range(N_CORES)))
            break
        except Exception:
            # transient NRT_EXEC_UNIT_UNRECOVERABLE wedges can follow an
            # earlier crashed process on the same cores; retry clears it
            if attempt == 2:
                raise
            import time

            time.sleep(5.0)
    outT = np.concatenate(
        [res.results[c]["out"] for c in range(N_CORES)], axis=0
    )  # (OUT, M*B)
    return asc(outT.T.astype(np.float32))  # (M*B, OUT)
